# revision 24
# baseline (speedup 1.0000x reference)
"""Trainium2 Bass kernel for nn_DiscreteStateTransition (NRI-style GNN message passing).

Reference computation (per batch b, time t):
  inputs[o]   = concat(x[b,o,t,:56], forward_probs[b,o,t,:8])          # [8, 64]
  pre_msg[e]  = concat(inputs[recv(e)], inputs[send(e)])               # [56, 128]
  h1          = relu(pre_msg @ W1 + b1)                                # [56, 512]
  msg         = relu(h1 @ W2 + b2)                                     # [56, 512]
  agg[o]      = sum over edges e with recv(e)==o of msg[e]             # [8, 512]
  out[o]      = concat(inputs[o], agg[o]) @ Wn + bn                    # [8, 64]

Sharding: data-parallel over (B=4) x (T-halves=2) -> 8 cores. Each core owns one
(b, t-half) slice: [8 objects, 256 timesteps]. Weights replicated.

On-chip layout is feature-major ("transposed"): features on SBUF partitions,
(node, time) / (recv, send, time) flattened along the free axis. Edges are
ordered recv-major so that the edge->node aggregation is a sum over groups of
7 adjacent (send) columns.
"""

import numpy as np

import concourse.bacc as bacc
import concourse.mybir as mybir
import concourse.tile as tile
from concourse.bass_utils import run_bass_kernel_spmd
from concourse.masks import make_identity

F32 = mybir.dt.float32

# Matmul operand dtype: float32r streams fp32 data through the PE in a single
# pass (4x faster than the 2-pass fp32 mode) at reduced multiply precision.
MM_DT = mybir.dt.float32r

# Problem constants (hardcoded per the harness contract).
B, O, T = 4, 8, 512
D = 64            # node feature size (56 + 8)
E = 56            # directed edges = O*(O-1)
H = 512           # msg hidden/out size
NET_IN = 576      # D + H
KK = 64           # K*K output features
TC = 256          # timesteps per core
TB = 32           # timesteps per chunk
NCHUNK = TC // TB
CE = E * TB       # edge columns per chunk (1792)
NN = O * TB       # node columns per chunk (256)
CB = 448          # matmul column block (= 2 recv groups x 7 x TB)
NCB = CE // CB    # 4


def build_nc(mm_dt=MM_DT, repeat=1):
    """Build the per-core Bass program (same program on all 8 cores)."""
    nc = bacc.Bacc("TRN2", target_bir_lowering=False, debug=False)

    xs = nc.dram_tensor("xs", [O, TC, 56], F32, kind="ExternalInput").ap()
    fps = nc.dram_tensor("fps", [O, TC, 8], F32, kind="ExternalInput").ap()
    w1 = nc.dram_tensor("w1", [2 * D, H], F32, kind="ExternalInput").ap()
    b1 = nc.dram_tensor("b1", [H], F32, kind="ExternalInput").ap()
    w2 = nc.dram_tensor("w2", [H, H], F32, kind="ExternalInput").ap()
    b2 = nc.dram_tensor("b2", [H], F32, kind="ExternalInput").ap()
    wn = nc.dram_tensor("wn", [NET_IN, KK], F32, kind="ExternalInput").ap()
    bn = nc.dram_tensor("bn", [KK], F32, kind="ExternalInput").ap()
    out = nc.dram_tensor("out", [O, TC, KK], F32, kind="ExternalOutput").ap()

    AF = mybir.ActivationFunctionType
    ALU = mybir.AluOpType
    # SBUF tensors consumed by matmuls must carry the matmul dtype so their
    # producers round on write (walrus rejects unrounded fp32r inputs).
    MD = mm_dt

    with tile.TileContext(nc) as tc:
        with (
            tc.tile_pool(name="const", bufs=1) as const,
            tc.tile_pool(name="inp", bufs=3) as inp_pool,
            tc.tile_pool(name="pre", bufs=3) as pre_pool,
            tc.tile_pool(name="h1p", bufs=2) as h1_pool,
            tc.tile_pool(name="msgp", bufs=6) as msg_pool,
            tc.tile_pool(name="aggp", bufs=2) as agg_pool,
            tc.tile_pool(name="tmpp", bufs=2) as tmp_pool,
            tc.tile_pool(name="netp", bufs=2) as net_pool,
            tc.tile_pool(name="orm", bufs=3) as orm_pool,
            tc.tile_pool(name="l1ps", bufs=3, space="PSUM") as l1ps,
            tc.tile_pool(name="l2ps", bufs=2, space="PSUM") as l2ps,
            tc.tile_pool(name="opps", bufs=1, space="PSUM") as opps,
        ):
            # ---- constants / weights ----
            ident = const.tile([128, 128], F32)
            make_identity(nc, ident)

            # weights: DMA into fp32 staging, round-copy into matmul-dtype tiles
            w1s = const.tile([128, H], F32)
            nc.gpsimd.dma_start(w1s[:], w1)
            w2s = const.tile([128, 4 * H], F32)
            for k in range(4):
                nc.gpsimd.dma_start(w2s[:, k * H:(k + 1) * H], w2[k * 128:(k + 1) * 128, :])
            wns = const.tile([128, 5 * KK], F32)
            nc.gpsimd.dma_start(wns[0:64, 4 * KK:5 * KK], wn[0:64, :])
            for k in range(4):
                nc.gpsimd.dma_start(wns[:, k * KK:(k + 1) * KK],
                                  wn[64 + k * 128:64 + (k + 1) * 128, :])
            w1t = const.tile([128, H], MD)
            nc.scalar.copy(w1t[:], w1s[:])
            w2t = const.tile([128, 4 * H], MD)
            nc.scalar.copy(w2t[:], w2s[:])
            wnin = const.tile([64, KK], MD)          # Wn rows 0:64 (node-input part)
            nc.scalar.copy(wnin[:], wns[0:64, 4 * KK:5 * KK])
            wnagg = const.tile([128, 4 * KK], MD)    # Wn rows 64+128k
            nc.scalar.copy(wnagg[:], wns[:, 0:4 * KK])
            b1t = const.tile([128, 4], F32)
            nc.gpsimd.dma_start(b1t[:], b1.rearrange("(f p) -> p f", p=128))
            b2t = const.tile([128, 4], F32)
            nc.gpsimd.dma_start(b2t[:], b2.rearrange("(f p) -> p f", p=128))
            bnt = const.tile([64, 1], F32)
            nc.gpsimd.dma_start(bnt[:], bn.unsqueeze(1))

            # ---- load node features, transpose to feature-major ----
            # inputsT[:, o*TC + t] = concat(x, fp)[o, t, :]. t-half 0 is
            # loaded+transposed first (chunk 0 needs it); t-half 1 transposes
            # are deferred as thunks interleaved into chunk 0's layer 1.
            inputsT = const.tile([64, O * TC], MD)
            rms = {}
            for th in range(2):
                for o in range(O):
                    rm = inp_pool.tile([128, 64], F32, name=f"rm{th}_{o}", tag="rm")
                    nc.sync.dma_start(rm[:, 0:56], xs[o, th * 128:(th + 1) * 128, :])
                    nc.sync.dma_start(rm[:, 56:64], fps[o, th * 128:(th + 1) * 128, :])
                    rms[(th, o)] = rm

            def make_intr(th, o):
                def unit():
                    rm = rms[(th, o)]
                    tp = opps.tile([64, 128], F32, name="tp", tag="op")
                    nc.tensor.transpose(tp[:], rm[:], ident[:])
                    cb0 = o * TC + th * 128
                    nc.scalar.copy(inputsT[:, cb0:cb0 + 128], tp[:])
                return unit

            for o in range(O):
                make_intr(0, o)()
            # Without a hardware loop the t-half-1 transposes can be deferred
            # into chunk 0's layer-1 stalls; under For_i they must stay outside
            # the loop body (back-edge semaphore reset would deadlock on their
            # pre-loop producers).
            input_tq = []
            if repeat > 1:
                for o in range(O):
                    make_intr(1, o)()
            else:
                input_tq = [make_intr(1, o) for o in range(O)]

            inT = inputsT.rearrange("p (o t) -> p o t", o=O)

            import contextlib
            loop_ctx = (tc.For_i(0, repeat, 1) if repeat > 1
                        else contextlib.nullcontext())
            with loop_ctx:
                pend_l2 = None    # L2 unit thunks from the previous chunk
                pend_agg = None   # aggregation thunks from the previous chunk
                pend_heads = []   # output-head thunks (deferred two chunks)

                def make_head(t0, inc, aggT):
                    def head():
                        np_ps = opps.tile([64, NN], F32, name="np_ps", tag="op")
                        nc.tensor.matmul(np_ps[:], wnin[:], inc,
                                         start=True, stop=False)
                        for k in range(4):
                            nc.tensor.matmul(
                                np_ps[:],
                                wnagg[:, k * KK:(k + 1) * KK],
                                aggT[:, k, :],
                                start=False, stop=(k == 3))
                        netoutT = net_pool.tile([64, NN], F32, name="netoutT")
                        nc.scalar.activation(netoutT[:], np_ps[:], AF.Identity,
                                             bias=bnt[:])
                        for hf in range(2):
                            tp2 = opps.tile([128, 64], F32, name="tp2", tag="op")
                            nc.tensor.transpose(
                                tp2[:], netoutT[:, hf * 128:(hf + 1) * 128],
                                ident[0:64, 0:64])
                            outrm = orm_pool.tile([128, 64], F32, name="outrm")
                            nc.vector.tensor_copy(outrm[:], tp2[:])
                            o0 = hf * 4
                            nc.sync.dma_start(
                                out[o0:o0 + 4, t0:t0 + TB, :],
                                outrm[:, :])
                    return head

                for c in range(NCHUNK):
                    t0 = c * TB
                    inc = inT[:, :, t0:t0 + TB]          # [64, 8, TB]

                    # ---- build pre_msgT [128, (r, s, t)] ----
                    pre = pre_pool.tile([128, CE], MD, name="pre")
                    pre_r = pre[0:64, :].rearrange("p (r s t) -> p r s t", r=O, s=O - 1)
                    pre_s = pre[64:128, :].rearrange("p (r s t) -> p r s t", r=O, s=O - 1)
                    # recv half: broadcast node r over its 7 outgoing slots
                    nc.gpsimd.tensor_copy(
                        pre_r, inc.unsqueeze(2).broadcast_to([64, O, O - 1, TB]))
                    # send half: for group r, senders are 0..r-1, r+1..7
                    for r in range(O):
                        if r > 0:
                            nc.gpsimd.tensor_copy(pre_s[:, r, 0:r, :], inc[:, 0:r, :])
                        if r < O - 1:
                            nc.gpsimd.tensor_copy(pre_s[:, r, r:, :], inc[:, r + 1:, :])

                    # ---- layer 1 units (MM + relu-evict), interleaved with the
                    # previous chunk's layer-2 units so the in-order PE always
                    # has ready matmuls while L1 PSUM slots recycle ----
                    h1 = h1_pool.tile([128, 4, CE], MD, name="h1")

                    def make_l1(cb, f1, h1=h1, pre=pre):
                        def unit():
                            hp = l1ps.tile([128, CB], F32, name="hp", tag="l1")
                            nc.tensor.matmul(
                                hp[:],
                                w1t[:, f1 * 128:(f1 + 1) * 128],
                                pre[:, cb * CB:(cb + 1) * CB],
                                start=True, stop=True)
                            if (f1 * NCB + cb) % 16 < 6:
                                nc.scalar.activation(
                                    h1[:, f1, cb * CB:(cb + 1) * CB], hp[:],
                                    AF.Relu, bias=b1t[:, f1:f1 + 1])
                            else:
                                nc.vector.tensor_scalar(
                                    out=h1[:, f1, cb * CB:(cb + 1) * CB], in0=hp[:],
                                    scalar1=b1t[:, f1:f1 + 1], scalar2=0.0,
                                    op0=ALU.add, op1=ALU.max)
                        return unit

                    l1_units = [make_l1(cb, f1) for cb in range(NCB) for f1 in range(4)]
                    l2q = list(pend_l2) if pend_l2 else []
                    for i, u in enumerate(l1_units):
                        u()
                        if i % 2 == 1 and l2q:
                            l2q.pop(0)()
                        elif input_tq:
                            input_tq.pop(0)()
                    for u in l2q:
                        u()
                    if pend_agg:
                        for u in pend_agg:
                            u()
                    if len(pend_heads) >= 2:
                        pend_heads.pop(0)()

                    # ---- build this chunk's layer-2 / aggregation / head ----
                    aggT = agg_pool.tile([128, 4, NN], MD, name="aggT")
                    msgs = [msg_pool.tile([128, CE], F32, name=f"msg{f2}", tag="msg")
                            for f2 in range(4)]

                    def make_l2(half, f2, h1=h1, msgs=msgs):
                        def unit():
                            msg = msgs[f2]
                            mp = l2ps.tile([128, 2, 512], F32, name="mp", tag="l2")
                            for k in range(4):
                                for cbh in range(2):
                                    cb = half * 2 + cbh
                                    nc.tensor.matmul(
                                        mp[:, cbh, 0:CB],
                                        w2t[:, k * H + f2 * 128:k * H + (f2 + 1) * 128],
                                        h1[:, k, cb * CB:(cb + 1) * CB],
                                        start=(k == 0), stop=(k == 3))
                            nc.scalar.activation(
                                msg[:, half * 2 * CB:(half + 1) * 2 * CB].rearrange(
                                    "p (c x) -> p c x", c=2),
                                mp[:, :, 0:CB],
                                AF.Relu, bias=b2t[:, f2:f2 + 1])
                        return unit

                    def make_agg(f2, c=c, msgs=msgs, aggT=aggT):
                        def unit():
                            # sum the 7 send-columns of each (r, t) group; one
                            # gpsimd add-tree unit per chunk, the rest on DVE
                            msg = msgs[f2]
                            mg = msg.rearrange("p (r s t) -> p r s t", r=O, s=O - 1)
                            agg_dst = aggT[:, f2, :].rearrange("p (r t) -> p r t", r=O)
                            if f2 == 0:
                                tm1 = tmp_pool.tile([128, O, 3, TB], F32, name="tm1")
                                nc.gpsimd.tensor_add(tm1[:], mg[:, :, 0:6:2, :], mg[:, :, 1:7:2, :])
                                tm2 = tmp_pool.tile([128, O, TB], F32, name="tm2")
                                nc.gpsimd.tensor_add(tm2[:], tm1[:, :, 0, :], tm1[:, :, 1, :])
                                tm3 = tmp_pool.tile([128, O, TB], F32, name="tm3")
                                nc.gpsimd.tensor_add(tm3[:], tm2[:], tm1[:, :, 2, :])
                                nc.gpsimd.tensor_add(agg_dst, tm3[:], mg[:, :, 6, :])
                            else:
                                with nc.allow_low_precision(reason="fp32r rounding on write"):
                                    nc.vector.reduce_sum(
                                        agg_dst,
                                        msg.rearrange("p (r s t) -> p r t s", r=O, s=O - 1),
                                        axis=mybir.AxisListType.X)
                        return unit

                    pend_l2 = [make_l2(half, f2) for half in range(2) for f2 in range(4)]
                    pend_agg = [make_agg(f2) for f2 in range(4)]
                    pend_heads.append(make_head(t0, inc, aggT))

                # ---- drain the software pipeline ----
                if pend_l2:
                    for u in pend_l2:
                        u()
                if pend_agg:
                    for u in pend_agg:
                        u()
                for hthunk in pend_heads:
                    hthunk()

    nc.compile()
    return nc


_NC_CACHE = {}


def _get_nc():
    key = (MM_DT, 1)
    if key not in _NC_CACHE:
        _NC_CACHE[key] = build_nc(MM_DT, 1)
    return _NC_CACHE[key]


def shard_inputs(x, forward_probs, **_):
    x = np.ascontiguousarray(np.asarray(x, dtype=np.float32))
    fp = np.ascontiguousarray(np.asarray(forward_probs, dtype=np.float32))
    in_maps = []
    for c in range(8):
        b, th = c // 2, c % 2
        in_maps.append({
            "xs": np.ascontiguousarray(x[b, :, th * TC:(th + 1) * TC, :]),
            "fps": np.ascontiguousarray(fp[b, :, th * TC:(th + 1) * TC, :]),
        })
    return in_maps


def kernel(y, x, hidden_states, forward_probs, edge_est, edge_gt,
           W1, b1, W2, b2, Wn, bn, edge2node):
    nc = _get_nc()
    weights = {
        "w1": np.ascontiguousarray(np.asarray(W1, dtype=np.float32)),
        "b1": np.ascontiguousarray(np.asarray(b1, dtype=np.float32)),
        "w2": np.ascontiguousarray(np.asarray(W2, dtype=np.float32)),
        "b2": np.ascontiguousarray(np.asarray(b2, dtype=np.float32)),
        "wn": np.ascontiguousarray(np.asarray(Wn, dtype=np.float32)),
        "bn": np.ascontiguousarray(np.asarray(bn, dtype=np.float32)),
    }
    in_maps = [dict(m, **weights) for m in shard_inputs(x, forward_probs)]
    res = run_bass_kernel_spmd(nc, in_maps, list(range(8)))
    full = np.empty((B, O, T, KK), dtype=np.float32)
    for c in range(8):
        b, th = c // 2, c % 2
        full[b, :, th * TC:(th + 1) * TC, :] = res.results[c]["out"]
    return full.reshape(B, O, T, 8, 8)
